# revision 35
# baseline (speedup 1.0000x reference)
import sys

sys.path.insert(0, "/opt/trn_rl_repo")

import numpy as np
import ml_dtypes

import concourse.bass as bass
import concourse.bacc as bacc
import concourse.mybir as mybir
import concourse.tile as tile
from concourse import library_config
from concourse.bass_utils import run_bass_kernel_spmd

BF16 = mybir.dt.float16  # fp16: same PE/DVE speed as bf16, 8x finer mantissa
F32 = mybir.dt.float32
AF = mybir.ActivationFunctionType
ALU = mybir.AluOpType

B, N, CD, GD, NH = 32, 512, 80, 50, 3  # batch, nodes, comp_dim, gat_dim, heads
NC_ = 8            # cores
MPC = B // NC_     # molecules per core = 4
NCH = N // 128     # 128-partition chunks per N = 4
FAo = CD + 1       # out-layer lhsT cols: 80 Wh + 1 e2col
NEG = -30000.0     # mask offset (exp underflows to 0 after lrelu)


def _scal_chunk(m, li, c):
    """Which (molecule, layer, chunk) runs leaky-relu on the scalar engine
    (Prelu with fused e2col bias) instead of the vector engine."""
    return not (c == 1 and (m + li) % 2 == 0)


def _build_nc():
    nc = bacc.Bacc("TRN2", target_bir_lowering=False, debug=False, num_devices=NC_)

    hTb_d = nc.dram_tensor("hTb", [CD, MPC, N], BF16, kind="ExternalInput")
    lm_d = nc.dram_tensor("lm", [128, MPC, NCH, N], BF16, kind="ExternalInput")
    Wcomb_d = nc.dram_tensor("Wcomb", [CD, NH * GD + NH], BF16, kind="ExternalInput")
    wa1_d = nc.dram_tensor("wa1", [CD, NH], BF16, kind="ExternalInput")
    WoutE_d = nc.dram_tensor("WoutE", [GD + 1, NH, FAo], BF16, kind="ExternalInput")
    waout1_d = nc.dram_tensor("waout1", [GD + 1, NH], BF16, kind="ExternalInput")
    negcsum_d = nc.dram_tensor("negcsum", [1, FAo], BF16, kind="ExternalInput")
    out_d = nc.dram_tensor("out", [MPC, FAo + 1, N], F32, kind="ExternalOutput")

    with tile.TileContext(nc) as tc:
        with (
            tc.tile_pool(name="persist", bufs=1) as pp,
            tc.tile_pool(name="mpool", bufs=3) as mp,    # per-molecule tiles
            tc.tile_pool(name="hpool", bufs=3) as hp,    # per-head tiles
            tc.tile_pool(name="psW", bufs=1, space="PSUM") as psW,
            tc.tile_pool(name="psWo", bufs=1, space="PSUM") as psWo,
            tc.tile_pool(name="psE", bufs=1, space="PSUM") as psE,
            tc.tile_pool(name="psO", bufs=3, space="PSUM") as psO,
            tc.tile_pool(name="psOo", bufs=1, space="PSUM") as psOo,
        ):
            nc.gpsimd.load_library(library_config.attn)

            # ---- persistent staging ----
            hTb_s = pp.tile([CD, MPC, N], BF16, tag="hTb")
            nc.sync.dma_start(hTb_s[:], hTb_d[:])
            lm_s = pp.tile([128, MPC, NCH, N], BF16, tag="lm")
            for m in range(MPC):
                nc.sync.dma_start(lm_s[:, m], lm_d[:, m])
            Wcomb_s = pp.tile([CD, NH * GD + NH], BF16, tag="Wcomb")
            nc.sync.dma_start(Wcomb_s[:], Wcomb_d[:])
            wa1_s = pp.tile([CD, NH], BF16, tag="wa1")
            nc.sync.dma_start(wa1_s[:], wa1_d[:])
            WoutE_s = pp.tile([GD + 1, NH, FAo], BF16, tag="WoutE")
            nc.sync.dma_start(WoutE_s[:], WoutE_d[:])
            waout1_s = pp.tile([GD + 1, NH], BF16, tag="waout1")
            nc.sync.dma_start(waout1_s[:], waout1_d[:])
            negcsum_s = pp.tile([1, FAo], BF16, tag="negcsum")
            nc.sync.dma_start(negcsum_s[:], negcsum_d[:])
            ones1_s = pp.tile([1, 128], BF16, tag="ones1")
            nc.vector.memset(ones1_s[:], 1.0)

            def attention(m, li, E1b, e2col_ap_fn, whaug_ap_fn, OT, tagp):
                """Attention chunk pipeline. OT row 0 accumulates the softmax
                denominator (lhsT col 0 is ones); rows 1.. are features."""
                Elm = hp.tile([128, NCH, N], BF16, tag=f"Elm{tagp}")
                Lt = hp.tile([128, NCH, N], BF16, tag=f"Lt{tagp}")
                Pt = hp.tile([128, NCH, N], BF16, tag=f"Pt{tagp}")
                nc.vector.tensor_tensor(
                    Elm[:], E1b[:, None, :].broadcast_to([128, NCH, N]),
                    lm_s[:, m], op=ALU.add,
                )
                for c in range(NCH):
                    if _scal_chunk(m, li, c):
                        nc.scalar.activation(
                            Lt[:, c], Elm[:, c], AF.Prelu,
                            bias=e2col_ap_fn(c), scale=1.0, alpha=0.2,
                        )
                    else:
                        Dt = hp.tile([128, N], BF16, tag=f"Dt{tagp}")
                        D5 = hp.tile([128, N], BF16, tag=f"D5{tagp}")
                        nc.vector.tensor_scalar(
                            Dt[:], Elm[:, c], e2col_ap_fn(c), None, op0=ALU.add
                        )
                        nc.vector.tensor_scalar(
                            D5[:], Dt[:], 0.2, None, op0=ALU.mult
                        )
                        nc.vector.tensor_tensor(
                            Lt[:, c], Dt[:], D5[:], op=ALU.max
                        )
                nc.scalar.activation(Pt[:], Lt[:], AF.Exp)
                for c in range(NCH):
                    nc.tensor.matmul(
                        OT[:], whaug_ap_fn(c), Pt[:, c],
                        start=(c == 0), stop=(c == NCH - 1),
                    )

            def head_stage(m):
                whaug = mp.tile([128, NCH, NH, 52], BF16, tag="whaug")
                nc.vector.memset(whaug[:, :, :, 0:1], 1.0)
                e2colb = mp.tile([128, NCH, NH], F32, tag="e2colb")
                for cp in range(NCH // 2):
                    whn_p = psW.tile([128, 2, NH * GD + NH], F32, tag="whn")
                    for ci in range(2):
                        c = cp * 2 + ci
                        nc.tensor.matmul(
                            whn_p[:, ci], hTb_s[:, m, c * 128:(c + 1) * 128],
                            Wcomb_s[:], start=True, stop=True,
                        )
                    nc.vector.tensor_copy(
                        whaug[:, cp * 2:cp * 2 + 2, :, 1:51],
                        whn_p[:, :, 0:NH * GD].rearrange(
                            "p t (h g) -> p t h g", h=NH
                        ),
                    )
                    nc.vector.tensor_copy(
                        e2colb[:, cp * 2:cp * 2 + 2], whn_p[:, :, NH * GD:]
                    )

                e1b_sb = mp.tile([1, NH, N], BF16, tag="e1b_sb")
                for h in range(NH):
                    eph = psE.tile([1, N], F32, tag="ep")
                    nc.tensor.matmul(
                        eph[:], wa1_s[:, h:h + 1], hTb_s[:, m],
                        start=True, stop=True,
                    )
                    nc.vector.tensor_copy(e1b_sb[0:1, h], eph[:])

                hts = mp.tile([GD + 1, NH, N], BF16, tag="hts")
                OTs_h = []
                for h in range(NH):
                    e1bp = psE.tile([128, N], F32, tag="e1bp")
                    nc.tensor.matmul(
                        e1bp[:], ones1_s[:], e1b_sb[0:1, h], start=True, stop=True
                    )
                    E1b = hp.tile([128, N], BF16, tag="E1b")
                    nc.vector.tensor_copy(E1b[:], e1bp[:])
                    OT = psO.tile([GD + 1, N], F32, tag="OTh")
                    attention(
                        m, h, E1b,
                        lambda c: e2colb[:, c, h:h + 1],
                        lambda c: whaug[:, c, h, 0:51],
                        OT, "h",
                    )
                    OTs_h.append(OT)
                Yb = mp.tile([GD + 1, NH, N], BF16, tag="Yb")
                Mn = mp.tile([GD + 1, NH, N], BF16, tag="Mn")
                for h in range(NH):
                    # normalize: Y = OT * (1/den), den in OT row 0
                    OT = OTs_h[h]
                    Rf = hp.tile([1, N], F32, tag="Rf")
                    nc.vector.reciprocal_approx_fast(Rf[:], OT[0:1])
                    DenB = hp.tile([GD + 1, N], F32, tag="DenB")
                    nc.gpsimd.partition_broadcast(DenB[:], Rf[:])
                    nc.vector.tensor_tensor(Yb[:, h], OT[:], DenB[:], op=ALU.mult)
                    nc.vector.tensor_scalar(Mn[:, h], Yb[:, h], 0.0, None, op0=ALU.min)
                # ELU(+1): hts = relu(Y) + exp(min(Y,0)), exp batched over heads
                Ee = mp.tile([GD + 1, NH, N], BF16, tag="Ee")
                nc.scalar.activation(Ee[:], Mn[:], AF.Exp)
                Rl = mp.tile([GD + 1, NH, N], BF16, tag="Rl")
                nc.vector.tensor_scalar(Rl[:], Yb[:], 0.0, None, op0=ALU.max)
                nc.vector.tensor_tensor(hts[:], Rl[:], Ee[:], op=ALU.add)
                return hts

            def out_stage(m, hts):
                whaugO = mp.tile([128, NCH, 82], BF16, tag="whaugO")
                nc.vector.memset(whaugO[:, :, 0:1], 1.0)
                e2colbO = mp.tile([128, NCH, 1], F32, tag="e2colbO")
                for cp in range(NCH // 2):
                    wo_p = psWo.tile([128, 2, FAo], F32, tag="wo")
                    for ci in range(2):
                        c = cp * 2 + ci
                        for h in range(NH):
                            nc.tensor.matmul(
                                wo_p[:, ci], hts[:, h, c * 128:(c + 1) * 128],
                                WoutE_s[:, h], start=(h == 0), stop=False,
                            )
                        nc.tensor.matmul(
                            wo_p[:, ci], ones1_s[:], negcsum_s[:],
                            start=False, stop=True,
                        )
                    nc.vector.tensor_copy(
                        whaugO[:, cp * 2:cp * 2 + 2, 1:81], wo_p[:, :, 0:80]
                    )
                    nc.vector.tensor_copy(
                        e2colbO[:, cp * 2:cp * 2 + 2], wo_p[:, :, 80:81]
                    )

                epo = psE.tile([1, N], F32, tag="ep")
                for h in range(NH):
                    nc.tensor.matmul(
                        epo[:], waout1_s[:, h:h + 1], hts[:, h],
                        start=(h == 0), stop=(h == NH - 1),
                    )
                e1bo = mp.tile([1, N], BF16, tag="e1bo")
                nc.vector.tensor_copy(e1bo[:], epo[:])
                e1bop = psE.tile([128, N], F32, tag="e1bp")
                nc.tensor.matmul(
                    e1bop[:], ones1_s[:], e1bo[:], start=True, stop=True
                )
                E1bO = hp.tile([128, N], BF16, tag="E1bO")
                nc.vector.tensor_copy(E1bO[:], e1bop[:])

                OTo = psOo.tile([FAo + 1, N], F32, tag="OTo")
                attention(
                    m, NH, E1bO,
                    lambda c: e2colbO[:, c, 0:1],
                    lambda c: whaugO[:, c, 0:FAo + 1],
                    OTo, "o",
                )
                OTs = mp.tile([FAo + 1, N], F32, tag="OTs")
                nc.vector.tensor_copy(OTs[:], OTo[:])
                nc.sync.dma_start(out_d[m], OTs[:])

            # skewed software pipeline: molecule m's head stage issues before
            # molecule m-1's output stage, giving every engine independent
            # work to hide the serial output tail.
            hts_prev = None
            for m in range(MPC):
                hts_m = head_stage(m)
                if hts_prev is not None:
                    out_stage(m - 1, hts_prev)
                hts_prev = hts_m
            out_stage(MPC - 1, hts_prev)

    nc.compile()
    return nc


_NC_CACHE = None
_LAST_IN_MAPS = None


def kernel(h, adj, Ws, attn_a, W_out, a_out):
    global _NC_CACHE, _LAST_IN_MAPS
    h = np.asarray(h, dtype=np.float32)
    adj = np.asarray(adj)
    Ws = np.asarray(Ws, dtype=np.float32)
    attn_a = np.asarray(attn_a, dtype=np.float32)
    W_out = np.asarray(W_out, dtype=np.float32)
    a_out = np.asarray(a_out, dtype=np.float32)
    bf16 = np.float16

    # ---- replicated params ----
    W_all = Ws.transpose(1, 0, 2).reshape(CD, NH * GD)      # [80, 150]
    wa1 = np.stack([Ws[hh] @ attn_a[hh, :GD] for hh in range(NH)], axis=1)  # [80,3]
    wa2 = np.stack([Ws[hh] @ attn_a[hh, GD:] for hh in range(NH)], axis=1)  # [80,3]
    Wcomb = np.concatenate([W_all, wa2], axis=1).astype(bf16)  # [80, 153]

    waout1_f = W_out @ a_out[:CD]     # [150]
    waout2_f = W_out @ a_out[CD:]     # [150]
    # row 0 zero-padded: hts row 0 is the dummy den/den=1 channel
    WoutE = np.zeros((GD + 1, NH, FAo), dtype=np.float32)
    waout1 = np.zeros((GD + 1, NH), dtype=np.float32)
    for hh in range(NH):
        WoutE[1:, hh, 0:CD] = W_out[hh * GD:(hh + 1) * GD, :]
        WoutE[1:, hh, CD] = waout2_f[hh * GD:(hh + 1) * GD]
        waout1[1:, hh] = waout1_f[hh * GD:(hh + 1) * GD]
    WoutE = WoutE.astype(bf16)
    waout1 = waout1.astype(bf16)
    negcsum = np.zeros((1, FAo), dtype=np.float32)
    negcsum[0, 0:CD] = -W_out.sum(axis=0)
    # -c2 (ELU+1 shift of e2col) and -c0 (same shift of e1row, folded here
    # since both are constant offsets of the same logit sum)
    negcsum[0, CD] = -waout2_f.sum() - waout1_f.sum()
    negcsum = negcsum.astype(bf16)

    in_maps = []
    for k in range(NC_):
        mols = slice(k * MPC, (k + 1) * MPC)
        hT_core = np.ascontiguousarray(h[mols].transpose(2, 0, 1)).astype(bf16)
        a = adj[mols].transpose(0, 2, 1)                      # [4, j, i]
        a = a.reshape(MPC, NCH, 128, N).transpose(2, 0, 1, 3)  # [128,m,c,i]
        lm = np.where(a > 0, np.float32(0.0), np.float32(NEG)).astype(bf16)
        in_maps.append(
            {
                "hTb": hT_core,
                "lm": np.ascontiguousarray(lm),
                "Wcomb": Wcomb,
                "wa1": wa1.astype(bf16),
                "WoutE": WoutE,
                "waout1": waout1,
                "negcsum": negcsum,
            }
        )

    _LAST_IN_MAPS = in_maps
    if _NC_CACHE is None:
        _NC_CACHE = _build_nc()
    res = run_bass_kernel_spmd(_NC_CACHE, in_maps, core_ids=list(range(NC_)))
    outs = []
    for k in range(NC_):
        o = np.asarray(res.results[k]["out"], dtype=np.float32)  # [MPC, 82, N]
        den, num = o[:, 0:1, :], o[:, 1:1 + CD, :]
        outs.append((num / den).transpose(0, 2, 1))              # [MPC, N, CD]
    return np.concatenate(outs, axis=0).reshape(B, N, CD)


if __name__ == "__main__":
    import reference

    inputs = {k: np.asarray(v) for k, v in reference.setup_inputs().items()}
    exp = np.asarray(reference.reference(**inputs))
    got = kernel(**inputs)
    err = np.abs(got - exp).max() / (np.abs(exp).max() + 1e-9)
    print("Relative error:", err)


# revision 36
# speedup vs baseline: 1.0328x; 1.0328x over previous
import sys

sys.path.insert(0, "/opt/trn_rl_repo")

import numpy as np
import ml_dtypes

import concourse.bass as bass
import concourse.bacc as bacc
import concourse.mybir as mybir
import concourse.tile as tile
from concourse import library_config
from concourse.bass_utils import run_bass_kernel_spmd

BF16 = mybir.dt.float16  # fp16: same PE/DVE speed as bf16, 8x finer mantissa
F32 = mybir.dt.float32
AF = mybir.ActivationFunctionType
ALU = mybir.AluOpType

B, N, CD, GD, NH = 32, 512, 80, 50, 3  # batch, nodes, comp_dim, gat_dim, heads
NC_ = 8            # cores
MPC = B // NC_     # molecules per core = 4
NCH = N // 128     # 128-partition chunks per N = 4
FAo = CD + 1       # out-layer lhsT cols: 80 Wh + 1 e2col
NEG = -30000.0     # mask offset (exp underflows to 0 after lrelu)


def _scal_chunk(m, li, c):
    """Which (molecule, layer, chunk) runs leaky-relu on the scalar engine
    (Prelu with fused e2col bias) instead of the vector engine."""
    return not (c == 1 and (m + li) % 2 == 0)


def _build_nc():
    nc = bacc.Bacc("TRN2", target_bir_lowering=False, debug=False, num_devices=NC_)

    hTb_d = nc.dram_tensor("hTb", [CD, MPC, N], BF16, kind="ExternalInput")
    lm_d = nc.dram_tensor("lm", [128, MPC, NCH, N], BF16, kind="ExternalInput")
    Wcomb_d = nc.dram_tensor("Wcomb", [CD, NH * GD + NH], BF16, kind="ExternalInput")
    wa1_d = nc.dram_tensor("wa1", [CD, NH], BF16, kind="ExternalInput")
    WoutE_d = nc.dram_tensor("WoutE", [GD + 1, NH, FAo], BF16, kind="ExternalInput")
    waout1_d = nc.dram_tensor("waout1", [GD + 1, NH], BF16, kind="ExternalInput")
    negcsum_d = nc.dram_tensor("negcsum", [1, FAo], BF16, kind="ExternalInput")
    out_d = nc.dram_tensor("out", [MPC, FAo + 1, N], F32, kind="ExternalOutput")

    with tile.TileContext(nc) as tc:
        with (
            tc.tile_pool(name="persist", bufs=1) as pp,
            tc.tile_pool(name="mpool", bufs=3) as mp,    # per-molecule tiles
            tc.tile_pool(name="hpool", bufs=3) as hp,    # per-head tiles
            tc.tile_pool(name="psW", bufs=1, space="PSUM") as psW,
            tc.tile_pool(name="psWo", bufs=1, space="PSUM") as psWo,
            tc.tile_pool(name="psE", bufs=1, space="PSUM") as psE,
            tc.tile_pool(name="psO", bufs=3, space="PSUM") as psO,
            tc.tile_pool(name="psOo", bufs=1, space="PSUM") as psOo,
        ):
            nc.gpsimd.load_library(library_config.attn)

            # ---- persistent staging ----
            hTb_s = pp.tile([CD, MPC, N], BF16, tag="hTb")
            nc.sync.dma_start(hTb_s[:], hTb_d[:])
            lm_s = pp.tile([128, MPC, NCH, N], BF16, tag="lm")
            for m in range(MPC):
                nc.sync.dma_start(lm_s[:, m], lm_d[:, m])
            Wcomb_s = pp.tile([CD, NH * GD + NH], BF16, tag="Wcomb")
            nc.sync.dma_start(Wcomb_s[:], Wcomb_d[:])
            wa1_s = pp.tile([CD, NH], BF16, tag="wa1")
            nc.sync.dma_start(wa1_s[:], wa1_d[:])
            WoutE_s = pp.tile([GD + 1, NH, FAo], BF16, tag="WoutE")
            nc.sync.dma_start(WoutE_s[:], WoutE_d[:])
            waout1_s = pp.tile([GD + 1, NH], BF16, tag="waout1")
            nc.sync.dma_start(waout1_s[:], waout1_d[:])
            negcsum_s = pp.tile([1, FAo], BF16, tag="negcsum")
            nc.sync.dma_start(negcsum_s[:], negcsum_d[:])
            ones1_s = pp.tile([1, 128], BF16, tag="ones1")
            nc.vector.memset(ones1_s[:], 1.0)

            def attention(m, li, E1b, e2col_ap_fn, whaug_ap_fn, OT, tagp):
                """Attention chunk pipeline. OT row 0 accumulates the softmax
                denominator (lhsT col 0 is ones); rows 1.. are features."""
                Elm = hp.tile([128, NCH, N], BF16, tag=f"Elm{tagp}")
                Lt = hp.tile([128, NCH, N], BF16, tag=f"Lt{tagp}")
                Pt = hp.tile([128, NCH, N], BF16, tag=f"Pt{tagp}")
                nc.vector.tensor_tensor(
                    Elm[:], E1b[:, None, :].broadcast_to([128, NCH, N]),
                    lm_s[:, m], op=ALU.add,
                )
                for c in range(NCH):
                    if _scal_chunk(m, li, c):
                        nc.scalar.activation(
                            Lt[:, c], Elm[:, c], AF.Prelu,
                            bias=e2col_ap_fn(c), scale=1.0, alpha=0.2,
                        )
                    else:
                        Dt = hp.tile([128, N], BF16, tag=f"Dt{tagp}")
                        D5 = hp.tile([128, N], BF16, tag=f"D5{tagp}")
                        nc.vector.tensor_scalar(
                            Dt[:], Elm[:, c], e2col_ap_fn(c), None, op0=ALU.add
                        )
                        nc.vector.tensor_scalar(
                            D5[:], Dt[:], 0.2, None, op0=ALU.mult
                        )
                        nc.vector.tensor_tensor(
                            Lt[:, c], Dt[:], D5[:], op=ALU.max
                        )
                nc.scalar.activation(Pt[:], Lt[:], AF.Exp)
                for c in range(NCH):
                    nc.tensor.matmul(
                        OT[:], whaug_ap_fn(c), Pt[:, c],
                        start=(c == 0), stop=(c == NCH - 1),
                    )

            def head_stage(m):
                whaug = mp.tile([128, NCH, NH, 52], BF16, tag="whaug")
                nc.vector.memset(whaug[:, :, :, 0:1], 1.0)
                e2colb = mp.tile([128, NCH, NH], F32, tag="e2colb")
                for cp in range(NCH // 2):
                    whn_p = psW.tile([128, 2, NH * GD + NH], F32, tag="whn")
                    for ci in range(2):
                        c = cp * 2 + ci
                        nc.tensor.matmul(
                            whn_p[:, ci], hTb_s[:, m, c * 128:(c + 1) * 128],
                            Wcomb_s[:], start=True, stop=True,
                        )
                    nc.vector.tensor_copy(
                        whaug[:, cp * 2:cp * 2 + 2, :, 1:51],
                        whn_p[:, :, 0:NH * GD].rearrange(
                            "p t (h g) -> p t h g", h=NH
                        ),
                    )
                    nc.vector.tensor_copy(
                        e2colb[:, cp * 2:cp * 2 + 2], whn_p[:, :, NH * GD:]
                    )

                epp = psE.tile([NH, N], F32, tag="ep")
                nc.tensor.matmul(epp[:], wa1_s[:], hTb_s[:, m], start=True, stop=True)
                e1b_sb = mp.tile([NH, N], BF16, tag="e1b_sb")
                nc.vector.tensor_copy(e1b_sb[:], epp[:])
                e1cat = mp.tile([1, NH, N], BF16, tag="e1cat")
                nc.sync.dma_start(e1cat[:], e1b_sb[:])

                hts = mp.tile([GD + 1, NH, N], BF16, tag="hts")
                OTs_h = []
                for h in range(NH):
                    e1bp = psE.tile([128, N], F32, tag="e1bp")
                    nc.tensor.matmul(
                        e1bp[:], ones1_s[:], e1cat[0:1, h], start=True, stop=True
                    )
                    E1b = hp.tile([128, N], BF16, tag="E1b")
                    nc.vector.tensor_copy(E1b[:], e1bp[:])
                    OT = psO.tile([GD + 1, N], F32, tag="OTh")
                    attention(
                        m, h, E1b,
                        lambda c: e2colb[:, c, h:h + 1],
                        lambda c: whaug[:, c, h, 0:51],
                        OT, "h",
                    )
                    OTs_h.append(OT)
                Yb = mp.tile([GD + 1, NH, N], BF16, tag="Yb")
                Mn = mp.tile([GD + 1, NH, N], BF16, tag="Mn")
                for h in range(NH):
                    # normalize: Y = OT * (1/den), den in OT row 0
                    OT = OTs_h[h]
                    Rf = hp.tile([1, N], F32, tag="Rf")
                    nc.vector.reciprocal_approx_fast(Rf[:], OT[0:1])
                    DenB = hp.tile([GD + 1, N], F32, tag="DenB")
                    nc.gpsimd.partition_broadcast(DenB[:], Rf[:])
                    nc.vector.tensor_tensor(Yb[:, h], OT[:], DenB[:], op=ALU.mult)
                    nc.vector.tensor_scalar(Mn[:, h], Yb[:, h], 0.0, None, op0=ALU.min)
                # ELU(+1): hts = relu(Y) + exp(min(Y,0)), exp batched over heads
                Ee = mp.tile([GD + 1, NH, N], BF16, tag="Ee")
                nc.scalar.activation(Ee[:], Mn[:], AF.Exp)
                Rl = mp.tile([GD + 1, NH, N], BF16, tag="Rl")
                nc.vector.tensor_scalar(Rl[:], Yb[:], 0.0, None, op0=ALU.max)
                nc.vector.tensor_tensor(hts[:], Rl[:], Ee[:], op=ALU.add)
                return hts

            def out_stage(m, hts):
                whaugO = mp.tile([128, NCH, 82], BF16, tag="whaugO")
                nc.vector.memset(whaugO[:, :, 0:1], 1.0)
                e2colbO = mp.tile([128, NCH, 1], F32, tag="e2colbO")
                for cp in range(NCH // 2):
                    wo_p = psWo.tile([128, 2, FAo], F32, tag="wo")
                    for ci in range(2):
                        c = cp * 2 + ci
                        for h in range(NH):
                            nc.tensor.matmul(
                                wo_p[:, ci], hts[:, h, c * 128:(c + 1) * 128],
                                WoutE_s[:, h], start=(h == 0), stop=False,
                            )
                        nc.tensor.matmul(
                            wo_p[:, ci], ones1_s[:], negcsum_s[:],
                            start=False, stop=True,
                        )
                    nc.vector.tensor_copy(
                        whaugO[:, cp * 2:cp * 2 + 2, 1:81], wo_p[:, :, 0:80]
                    )
                    nc.vector.tensor_copy(
                        e2colbO[:, cp * 2:cp * 2 + 2], wo_p[:, :, 80:81]
                    )

                epo = psE.tile([1, N], F32, tag="ep")
                for h in range(NH):
                    nc.tensor.matmul(
                        epo[:], waout1_s[:, h:h + 1], hts[:, h],
                        start=(h == 0), stop=(h == NH - 1),
                    )
                e1bo = mp.tile([1, N], BF16, tag="e1bo")
                nc.vector.tensor_copy(e1bo[:], epo[:])
                e1bop = psE.tile([128, N], F32, tag="e1bp")
                nc.tensor.matmul(
                    e1bop[:], ones1_s[:], e1bo[:], start=True, stop=True
                )
                E1bO = hp.tile([128, N], BF16, tag="E1bO")
                nc.vector.tensor_copy(E1bO[:], e1bop[:])

                OTo = psOo.tile([FAo + 1, N], F32, tag="OTo")
                attention(
                    m, NH, E1bO,
                    lambda c: e2colbO[:, c, 0:1],
                    lambda c: whaugO[:, c, 0:FAo + 1],
                    OTo, "o",
                )
                OTs = mp.tile([FAo + 1, N], F32, tag="OTs")
                nc.vector.tensor_copy(OTs[:], OTo[:])
                nc.sync.dma_start(out_d[m], OTs[:])

            # skewed software pipeline: molecule m's head stage issues before
            # molecule m-1's output stage, giving every engine independent
            # work to hide the serial output tail.
            hts_prev = None
            for m in range(MPC):
                hts_m = head_stage(m)
                if hts_prev is not None:
                    out_stage(m - 1, hts_prev)
                hts_prev = hts_m
            out_stage(MPC - 1, hts_prev)

    nc.compile()
    return nc


_NC_CACHE = None
_LAST_IN_MAPS = None


def kernel(h, adj, Ws, attn_a, W_out, a_out):
    global _NC_CACHE, _LAST_IN_MAPS
    h = np.asarray(h, dtype=np.float32)
    adj = np.asarray(adj)
    Ws = np.asarray(Ws, dtype=np.float32)
    attn_a = np.asarray(attn_a, dtype=np.float32)
    W_out = np.asarray(W_out, dtype=np.float32)
    a_out = np.asarray(a_out, dtype=np.float32)
    bf16 = np.float16

    # ---- replicated params ----
    W_all = Ws.transpose(1, 0, 2).reshape(CD, NH * GD)      # [80, 150]
    wa1 = np.stack([Ws[hh] @ attn_a[hh, :GD] for hh in range(NH)], axis=1)  # [80,3]
    wa2 = np.stack([Ws[hh] @ attn_a[hh, GD:] for hh in range(NH)], axis=1)  # [80,3]
    Wcomb = np.concatenate([W_all, wa2], axis=1).astype(bf16)  # [80, 153]

    waout1_f = W_out @ a_out[:CD]     # [150]
    waout2_f = W_out @ a_out[CD:]     # [150]
    # row 0 zero-padded: hts row 0 is the dummy den/den=1 channel
    WoutE = np.zeros((GD + 1, NH, FAo), dtype=np.float32)
    waout1 = np.zeros((GD + 1, NH), dtype=np.float32)
    for hh in range(NH):
        WoutE[1:, hh, 0:CD] = W_out[hh * GD:(hh + 1) * GD, :]
        WoutE[1:, hh, CD] = waout2_f[hh * GD:(hh + 1) * GD]
        waout1[1:, hh] = waout1_f[hh * GD:(hh + 1) * GD]
    WoutE = WoutE.astype(bf16)
    waout1 = waout1.astype(bf16)
    negcsum = np.zeros((1, FAo), dtype=np.float32)
    negcsum[0, 0:CD] = -W_out.sum(axis=0)
    # -c2 (ELU+1 shift of e2col) and -c0 (same shift of e1row, folded here
    # since both are constant offsets of the same logit sum)
    negcsum[0, CD] = -waout2_f.sum() - waout1_f.sum()
    negcsum = negcsum.astype(bf16)

    in_maps = []
    for k in range(NC_):
        mols = slice(k * MPC, (k + 1) * MPC)
        hT_core = np.ascontiguousarray(h[mols].transpose(2, 0, 1)).astype(bf16)
        a = adj[mols].transpose(0, 2, 1)                      # [4, j, i]
        a = a.reshape(MPC, NCH, 128, N).transpose(2, 0, 1, 3)  # [128,m,c,i]
        lm = np.where(a > 0, np.float32(0.0), np.float32(NEG)).astype(bf16)
        in_maps.append(
            {
                "hTb": hT_core,
                "lm": np.ascontiguousarray(lm),
                "Wcomb": Wcomb,
                "wa1": wa1.astype(bf16),
                "WoutE": WoutE,
                "waout1": waout1,
                "negcsum": negcsum,
            }
        )

    _LAST_IN_MAPS = in_maps
    if _NC_CACHE is None:
        _NC_CACHE = _build_nc()
    res = run_bass_kernel_spmd(_NC_CACHE, in_maps, core_ids=list(range(NC_)))
    outs = []
    for k in range(NC_):
        o = np.asarray(res.results[k]["out"], dtype=np.float32)  # [MPC, 82, N]
        den, num = o[:, 0:1, :], o[:, 1:1 + CD, :]
        outs.append((num / den).transpose(0, 2, 1))              # [MPC, N, CD]
    return np.concatenate(outs, axis=0).reshape(B, N, CD)


if __name__ == "__main__":
    import reference

    inputs = {k: np.asarray(v) for k, v in reference.setup_inputs().items()}
    exp = np.asarray(reference.reference(**inputs))
    got = kernel(**inputs)
    err = np.abs(got - exp).max() / (np.abs(exp).max() + 1e-9)
    print("Relative error:", err)


# revision 37
# speedup vs baseline: 1.0531x; 1.0196x over previous
import sys

sys.path.insert(0, "/opt/trn_rl_repo")

import numpy as np
import ml_dtypes

import concourse.bass as bass
import concourse.bacc as bacc
import concourse.mybir as mybir
import concourse.tile as tile
from concourse import library_config
from concourse.bass_utils import run_bass_kernel_spmd

BF16 = mybir.dt.float16  # fp16: same PE/DVE speed as bf16, 8x finer mantissa
F32 = mybir.dt.float32
AF = mybir.ActivationFunctionType
ALU = mybir.AluOpType

B, N, CD, GD, NH = 32, 512, 80, 50, 3  # batch, nodes, comp_dim, gat_dim, heads
NC_ = 8            # cores
MPC = B // NC_     # molecules per core = 4
NCH = N // 128     # 128-partition chunks per N = 4
FAo = CD + 1       # out-layer lhsT cols: 80 Wh + 1 e2col
NEG = -30000.0     # mask offset (exp underflows to 0 after lrelu)


def _scal_chunk(m, li, c):
    """Which (molecule, layer, chunk) runs leaky-relu on the scalar engine
    (Prelu with fused e2col bias) instead of the vector engine."""
    return not (c == 1 and (m + li) % 2 == 0)


def _build_nc():
    nc = bacc.Bacc("TRN2", target_bir_lowering=False, debug=False, num_devices=NC_)

    hTb_d = nc.dram_tensor("hTb", [CD, MPC, N], BF16, kind="ExternalInput")
    lm_d = nc.dram_tensor("lm", [128, MPC, NCH, N], BF16, kind="ExternalInput")
    Wcomb_d = nc.dram_tensor("Wcomb", [CD, NH * GD + NH], BF16, kind="ExternalInput")
    wa1_d = nc.dram_tensor("wa1", [CD, NH], BF16, kind="ExternalInput")
    WoutE_d = nc.dram_tensor("WoutE", [GD + 1, NH, FAo], BF16, kind="ExternalInput")
    waout1_d = nc.dram_tensor("waout1", [GD + 1, NH], BF16, kind="ExternalInput")
    negcsum_d = nc.dram_tensor("negcsum", [1, FAo], BF16, kind="ExternalInput")
    out_d = nc.dram_tensor("out", [MPC, FAo + 1, N], F32, kind="ExternalOutput")

    with tile.TileContext(nc) as tc:
        with (
            tc.tile_pool(name="persist", bufs=1) as pp,
            tc.tile_pool(name="mpool", bufs=3) as mp,    # per-molecule tiles
            tc.tile_pool(name="hpool", bufs=3) as hp,    # per-head tiles
            tc.tile_pool(name="psW", bufs=1, space="PSUM") as psW,
            tc.tile_pool(name="psWo", bufs=1, space="PSUM") as psWo,
            tc.tile_pool(name="psE", bufs=1, space="PSUM") as psE,
            tc.tile_pool(name="psO", bufs=3, space="PSUM") as psO,
            tc.tile_pool(name="psOo", bufs=1, space="PSUM") as psOo,
        ):
            nc.gpsimd.load_library(library_config.attn)

            # ---- persistent staging ----
            hTb_s = pp.tile([CD, MPC, N], BF16, tag="hTb")
            nc.sync.dma_start(hTb_s[:], hTb_d[:])
            lm_s = pp.tile([128, MPC, NCH, N], BF16, tag="lm")
            for m in range(MPC):
                nc.sync.dma_start(lm_s[:, m], lm_d[:, m])
            Wcomb_s = pp.tile([CD, NH * GD + NH], BF16, tag="Wcomb")
            nc.sync.dma_start(Wcomb_s[:], Wcomb_d[:])
            wa1_s = pp.tile([CD, NH], BF16, tag="wa1")
            nc.sync.dma_start(wa1_s[:], wa1_d[:])
            WoutE_s = pp.tile([GD + 1, NH, FAo], BF16, tag="WoutE")
            nc.sync.dma_start(WoutE_s[:], WoutE_d[:])
            waout1_s = pp.tile([GD + 1, NH], BF16, tag="waout1")
            nc.sync.dma_start(waout1_s[:], waout1_d[:])
            negcsum_s = pp.tile([1, FAo], BF16, tag="negcsum")
            nc.sync.dma_start(negcsum_s[:], negcsum_d[:])
            ones1_s = pp.tile([1, 128], BF16, tag="ones1")
            nc.vector.memset(ones1_s[:], 1.0)

            def attention(m, li, E1b, e2col_ap_fn, whaug_ap_fn, OT, tagp):
                """Attention chunk pipeline. OT row 0 accumulates the softmax
                denominator (lhsT col 0 is ones); rows 1.. are features."""
                Elm = hp.tile([128, NCH, N], BF16, tag=f"Elm{tagp}")
                Lt = hp.tile([128, NCH, N], BF16, tag=f"Lt{tagp}")
                Pt = hp.tile([128, NCH, N], BF16, tag=f"Pt{tagp}")
                for half in range(2):
                    s = slice(half * 2, half * 2 + 2)
                    nc.vector.tensor_tensor(
                        Elm[:, s], E1b[:, None, :].broadcast_to([128, 2, N]),
                        lm_s[:, m, s], op=ALU.add,
                    )
                for c in range(NCH):
                    if _scal_chunk(m, li, c):
                        nc.scalar.activation(
                            Lt[:, c], Elm[:, c], AF.Prelu,
                            bias=e2col_ap_fn(c), scale=1.0, alpha=0.2,
                        )
                    else:
                        Dt = hp.tile([128, N], BF16, tag=f"Dt{tagp}")
                        D5 = hp.tile([128, N], BF16, tag=f"D5{tagp}")
                        nc.vector.tensor_scalar(
                            Dt[:], Elm[:, c], e2col_ap_fn(c), None, op0=ALU.add
                        )
                        nc.vector.tensor_scalar(
                            D5[:], Dt[:], 0.2, None, op0=ALU.mult
                        )
                        nc.vector.tensor_tensor(
                            Lt[:, c], Dt[:], D5[:], op=ALU.max
                        )
                nc.scalar.activation(Pt[:], Lt[:], AF.Exp)
                for c in range(NCH):
                    nc.tensor.matmul(
                        OT[:], whaug_ap_fn(c), Pt[:, c],
                        start=(c == 0), stop=(c == NCH - 1),
                    )

            def head_stage(m):
                whaug = mp.tile([128, NCH, NH, 52], BF16, tag="whaug")
                nc.vector.memset(whaug[:, :, :, 0:1], 1.0)
                e2colb = mp.tile([128, NCH, NH], F32, tag="e2colb")
                for cp in range(NCH // 2):
                    whn_p = psW.tile([128, 2, NH * GD + NH], F32, tag="whn")
                    for ci in range(2):
                        c = cp * 2 + ci
                        nc.tensor.matmul(
                            whn_p[:, ci], hTb_s[:, m, c * 128:(c + 1) * 128],
                            Wcomb_s[:], start=True, stop=True,
                        )
                    nc.vector.tensor_copy(
                        whaug[:, cp * 2:cp * 2 + 2, :, 1:51],
                        whn_p[:, :, 0:NH * GD].rearrange(
                            "p t (h g) -> p t h g", h=NH
                        ),
                    )
                    nc.vector.tensor_copy(
                        e2colb[:, cp * 2:cp * 2 + 2], whn_p[:, :, NH * GD:]
                    )

                epp = psE.tile([NH, N], F32, tag="ep")
                nc.tensor.matmul(epp[:], wa1_s[:], hTb_s[:, m], start=True, stop=True)
                e1b_sb = mp.tile([NH, N], BF16, tag="e1b_sb")
                nc.vector.tensor_copy(e1b_sb[:], epp[:])
                e1cat = mp.tile([1, NH, N], BF16, tag="e1cat")
                nc.sync.dma_start(e1cat[:], e1b_sb[:])

                hts = mp.tile([GD + 1, NH, N], BF16, tag="hts")
                OTs_h = []
                for h in range(NH):
                    e1bp = psE.tile([128, N], F32, tag="e1bp")
                    nc.tensor.matmul(
                        e1bp[:], ones1_s[:], e1cat[0:1, h], start=True, stop=True
                    )
                    E1b = hp.tile([128, N], BF16, tag="E1b")
                    nc.vector.tensor_copy(E1b[:], e1bp[:])
                    OT = psO.tile([GD + 1, N], F32, tag="OTh")
                    attention(
                        m, h, E1b,
                        lambda c: e2colb[:, c, h:h + 1],
                        lambda c: whaug[:, c, h, 0:51],
                        OT, "h",
                    )
                    OTs_h.append(OT)
                Yb = mp.tile([GD + 1, NH, N], BF16, tag="Yb")
                Mn = mp.tile([GD + 1, NH, N], BF16, tag="Mn")
                for h in range(NH):
                    # normalize: Y = OT * (1/den), den in OT row 0
                    OT = OTs_h[h]
                    Rf = hp.tile([1, N], F32, tag="Rf")
                    nc.vector.reciprocal_approx_fast(Rf[:], OT[0:1])
                    DenB = hp.tile([GD + 1, N], F32, tag="DenB")
                    nc.gpsimd.partition_broadcast(DenB[:], Rf[:])
                    nc.vector.tensor_tensor(Yb[:, h], OT[:], DenB[:], op=ALU.mult)
                    nc.vector.tensor_scalar(Mn[:, h], Yb[:, h], 0.0, None, op0=ALU.min)
                # ELU(+1): hts = relu(Y) + exp(min(Y,0)), exp batched over heads
                Ee = mp.tile([GD + 1, NH, N], BF16, tag="Ee")
                nc.scalar.activation(Ee[:], Mn[:], AF.Exp)
                Rl = mp.tile([GD + 1, NH, N], BF16, tag="Rl")
                nc.vector.tensor_scalar(Rl[:], Yb[:], 0.0, None, op0=ALU.max)
                nc.vector.tensor_tensor(hts[:], Rl[:], Ee[:], op=ALU.add)
                return hts

            def out_stage(m, hts):
                whaugO = mp.tile([128, NCH, 82], BF16, tag="whaugO")
                nc.vector.memset(whaugO[:, :, 0:1], 1.0)
                e2colbO = mp.tile([128, NCH, 1], F32, tag="e2colbO")
                for cp in range(NCH // 2):
                    wo_p = psWo.tile([128, 2, FAo], F32, tag="wo")
                    for ci in range(2):
                        c = cp * 2 + ci
                        for h in range(NH):
                            nc.tensor.matmul(
                                wo_p[:, ci], hts[:, h, c * 128:(c + 1) * 128],
                                WoutE_s[:, h], start=(h == 0), stop=False,
                            )
                        nc.tensor.matmul(
                            wo_p[:, ci], ones1_s[:], negcsum_s[:],
                            start=False, stop=True,
                        )
                    nc.vector.tensor_copy(
                        whaugO[:, cp * 2:cp * 2 + 2, 1:81], wo_p[:, :, 0:80]
                    )
                    nc.vector.tensor_copy(
                        e2colbO[:, cp * 2:cp * 2 + 2], wo_p[:, :, 80:81]
                    )

                epo = psE.tile([1, N], F32, tag="ep")
                for h in range(NH):
                    nc.tensor.matmul(
                        epo[:], waout1_s[:, h:h + 1], hts[:, h],
                        start=(h == 0), stop=(h == NH - 1),
                    )
                e1bo = mp.tile([1, N], BF16, tag="e1bo")
                nc.vector.tensor_copy(e1bo[:], epo[:])
                e1bop = psE.tile([128, N], F32, tag="e1bp")
                nc.tensor.matmul(
                    e1bop[:], ones1_s[:], e1bo[:], start=True, stop=True
                )
                E1bO = hp.tile([128, N], BF16, tag="E1bO")
                nc.vector.tensor_copy(E1bO[:], e1bop[:])

                OTo = psOo.tile([FAo + 1, N], F32, tag="OTo")
                attention(
                    m, NH, E1bO,
                    lambda c: e2colbO[:, c, 0:1],
                    lambda c: whaugO[:, c, 0:FAo + 1],
                    OTo, "o",
                )
                OTs = mp.tile([FAo + 1, N], F32, tag="OTs")
                nc.vector.tensor_copy(OTs[:], OTo[:])
                nc.sync.dma_start(out_d[m], OTs[:])

            # skewed software pipeline: molecule m's head stage issues before
            # molecule m-1's output stage, giving every engine independent
            # work to hide the serial output tail.
            hts_prev = None
            for m in range(MPC):
                hts_m = head_stage(m)
                if hts_prev is not None:
                    out_stage(m - 1, hts_prev)
                hts_prev = hts_m
            out_stage(MPC - 1, hts_prev)

    nc.compile()
    return nc


_NC_CACHE = None
_LAST_IN_MAPS = None


def kernel(h, adj, Ws, attn_a, W_out, a_out):
    global _NC_CACHE, _LAST_IN_MAPS
    h = np.asarray(h, dtype=np.float32)
    adj = np.asarray(adj)
    Ws = np.asarray(Ws, dtype=np.float32)
    attn_a = np.asarray(attn_a, dtype=np.float32)
    W_out = np.asarray(W_out, dtype=np.float32)
    a_out = np.asarray(a_out, dtype=np.float32)
    bf16 = np.float16

    # ---- replicated params ----
    W_all = Ws.transpose(1, 0, 2).reshape(CD, NH * GD)      # [80, 150]
    wa1 = np.stack([Ws[hh] @ attn_a[hh, :GD] for hh in range(NH)], axis=1)  # [80,3]
    wa2 = np.stack([Ws[hh] @ attn_a[hh, GD:] for hh in range(NH)], axis=1)  # [80,3]
    Wcomb = np.concatenate([W_all, wa2], axis=1).astype(bf16)  # [80, 153]

    waout1_f = W_out @ a_out[:CD]     # [150]
    waout2_f = W_out @ a_out[CD:]     # [150]
    # row 0 zero-padded: hts row 0 is the dummy den/den=1 channel
    WoutE = np.zeros((GD + 1, NH, FAo), dtype=np.float32)
    waout1 = np.zeros((GD + 1, NH), dtype=np.float32)
    for hh in range(NH):
        WoutE[1:, hh, 0:CD] = W_out[hh * GD:(hh + 1) * GD, :]
        WoutE[1:, hh, CD] = waout2_f[hh * GD:(hh + 1) * GD]
        waout1[1:, hh] = waout1_f[hh * GD:(hh + 1) * GD]
    WoutE = WoutE.astype(bf16)
    waout1 = waout1.astype(bf16)
    negcsum = np.zeros((1, FAo), dtype=np.float32)
    negcsum[0, 0:CD] = -W_out.sum(axis=0)
    # -c2 (ELU+1 shift of e2col) and -c0 (same shift of e1row, folded here
    # since both are constant offsets of the same logit sum)
    negcsum[0, CD] = -waout2_f.sum() - waout1_f.sum()
    negcsum = negcsum.astype(bf16)

    in_maps = []
    for k in range(NC_):
        mols = slice(k * MPC, (k + 1) * MPC)
        hT_core = np.ascontiguousarray(h[mols].transpose(2, 0, 1)).astype(bf16)
        a = adj[mols].transpose(0, 2, 1)                      # [4, j, i]
        a = a.reshape(MPC, NCH, 128, N).transpose(2, 0, 1, 3)  # [128,m,c,i]
        lm = np.where(a > 0, np.float32(0.0), np.float32(NEG)).astype(bf16)
        in_maps.append(
            {
                "hTb": hT_core,
                "lm": np.ascontiguousarray(lm),
                "Wcomb": Wcomb,
                "wa1": wa1.astype(bf16),
                "WoutE": WoutE,
                "waout1": waout1,
                "negcsum": negcsum,
            }
        )

    _LAST_IN_MAPS = in_maps
    if _NC_CACHE is None:
        _NC_CACHE = _build_nc()
    res = run_bass_kernel_spmd(_NC_CACHE, in_maps, core_ids=list(range(NC_)))
    outs = []
    for k in range(NC_):
        o = np.asarray(res.results[k]["out"], dtype=np.float32)  # [MPC, 82, N]
        den, num = o[:, 0:1, :], o[:, 1:1 + CD, :]
        outs.append((num / den).transpose(0, 2, 1))              # [MPC, N, CD]
    return np.concatenate(outs, axis=0).reshape(B, N, CD)


if __name__ == "__main__":
    import reference

    inputs = {k: np.asarray(v) for k, v in reference.setup_inputs().items()}
    exp = np.asarray(reference.reference(**inputs))
    got = kernel(**inputs)
    err = np.abs(got - exp).max() / (np.abs(exp).max() + 1e-9)
    print("Relative error:", err)
